# revision 1
# baseline (speedup 1.0000x reference)
"""MoE kernel for Trainium2 — 8-core expert-parallel + shared-expert 2D shard.

Strategy:
  - Host computes routing (replica of reference math, fp32) ONLY to decide
    data placement: which tokens go to which expert-core (top-2 dispatch).
    The combine weights used in the output math are recomputed ON DEVICE
    from raw inputs (centroid matmul in fp32 + sigmoid/top2/softmax).
  - Core e (e=0..7): routed expert e's MLP over its gathered tokens
    (capacity-padded to C, invalid rows masked to 0 on device), plus a
    (token-quarter x F-half) shard of the 2 shared experts.
  - Matmuls in float32r (full PE rate); routing matmul in exact float32.
  - Host unshard: scatter-add routed partials (unique indices per core),
    sum shared partials; residual x folded in on-device via x_res halves.
"""

import sys

sys.path.insert(0, "/opt/trn_rl_repo")

import numpy as np

D = 1024
F = 4096          # routed expert hidden
FSH = 4096        # shared shard hidden: 2 experts x (4096/2) F-half
E = 8
NS = 2
B, S = 2, 1024
TT = B * S        # 2048 tokens
TQ = TT // 4      # 512-token quarter per shared shard
FCH = 512         # hidden-chunk streamed per iteration

_prog_cache = {}


def _mchunks(n):
    """Split n into moving-dim chunks of 512 + remainder. Chunks must start
    at multiples of 512 so no matmul output crosses a PSUM bank boundary
    (fp32 bank = 512 floats)."""
    out = [512] * (n // 512)
    if n % 512:
        out.append(n % 512)
    return out


def _build(C):
    from contextlib import ExitStack
    from concourse import bacc, bass, tile, mybir

    f32 = mybir.dt.float32
    f32r = mybir.dt.float32r
    AF = mybir.ActivationFunctionType
    ALU = mybir.AluOpType
    AX = mybir.AxisListType

    nc = bacc.Bacc("TRN2", target_bir_lowering=False, debug=False, num_devices=8)

    d_xgT = nc.dram_tensor("xgT", [D, C], f32, kind="ExternalInput").ap()
    d_xqT = nc.dram_tensor("xqT", [D, TQ], f32r, kind="ExternalInput").ap()
    d_xres = nc.dram_tensor("x_res", [TQ, D], f32, kind="ExternalInput").ap()
    d_centT = nc.dram_tensor("centT", [D, E], f32, kind="ExternalInput").ap()
    d_rbias = nc.dram_tensor("rbias", [128, E], f32, kind="ExternalInput").ap()
    d_sel = nc.dram_tensor("sel", [128, E], f32, kind="ExternalInput").ap()
    d_valid = nc.dram_tensor("valid", [128, C // 128], f32, kind="ExternalInput").ap()
    d_ident = nc.dram_tensor("ident", [128, 128], f32, kind="ExternalInput").ap()
    d_wfc = nc.dram_tensor("wfcT", [D, F], f32r, kind="ExternalInput").ap()
    d_wpj = nc.dram_tensor("wprojT", [F, D], f32r, kind="ExternalInput").ap()
    d_wfcs = nc.dram_tensor("wfcshT", [D, FSH], f32r, kind="ExternalInput").ap()
    d_wpjs = nc.dram_tensor("wprojshT", [FSH, D], f32r, kind="ExternalInput").ap()
    d_outr = nc.dram_tensor("out_r", [C, D], f32, kind="ExternalOutput").ap()
    d_outs = nc.dram_tensor("out_sh", [TQ, D], f32, kind="ExternalOutput").ap()

    CJ = C // 128  # routed token tiles
    QJ = TQ // 128  # shared token tiles

    with tile.TileContext(nc) as tc, ExitStack() as ctx:
        const = ctx.enter_context(tc.tile_pool(name="const", bufs=1))
        xpool = ctx.enter_context(tc.tile_pool(name="xpool", bufs=1))
        ypool = ctx.enter_context(tc.tile_pool(name="ypool", bufs=1))
        rpool = ctx.enter_context(tc.tile_pool(name="rpool", bufs=2))
        wpool = ctx.enter_context(tc.tile_pool(name="wpool", bufs=2))
        hpool = ctx.enter_context(tc.tile_pool(name="hpool", bufs=2))

        # ---- resident loads -------------------------------------------------
        centT = const.tile([128, 8, E], f32)
        for di in range(8):
            nc.sync.dma_start(centT[:, di, :], d_centT[di * 128:(di + 1) * 128, :])
        rbias = const.tile([128, E], f32)
        nc.sync.dma_start(rbias[:], d_rbias[:, :])
        sel = const.tile([128, E], f32)
        nc.sync.dma_start(sel[:], d_sel[:, :])
        valid = const.tile([128, CJ], f32)
        nc.sync.dma_start(valid[:], d_valid[:, :])
        ident = const.tile([128, 128], f32)
        nc.sync.dma_start(ident[:], d_ident[:, :])
        zbias = const.tile([128, 1], f32)
        nc.vector.memset(zbias[:], 0.0)

        xgTr = xpool.tile([128, 8, C], f32r)
        xqT = xpool.tile([128, 8, TQ], f32r)
        for di in range(8):
            nc.sync.dma_start(xqT[:, di, :], d_xqT[di * 128:(di + 1) * 128, :])
        xres = xpool.tile([128, QJ, D], f32)
        for j in range(QJ):
            nc.sync.dma_start(xres[:, j, :], d_xres[j * 128:(j + 1) * 128, :])

        y_r = ypool.tile([128, CJ, D], f32)
        y_sh = ypool.tile([128, QJ, D], f32)
        ce = const.tile([128, CJ], f32)

        # ---- routing (fp32, device-side combine weights) --------------------
        with ExitStack() as rctx:
            xgf = rctx.enter_context(tc.tile_pool(name="xgf", bufs=1))
            rps = rctx.enter_context(tc.tile_pool(name="rps", bufs=1, space="PSUM"))
            tps = rctx.enter_context(tc.tile_pool(name="tps", bufs=2, space="PSUM"))
            xgT = xgf.tile([128, 8, C], f32)
            for di in range(8):
                nc.sync.dma_start(xgT[:, di, :], d_xgT[di * 128:(di + 1) * 128, :])
            # f32r-rounded copy for the MLP matmuls
            for di in range(8):
                nc.vector.tensor_copy(xgTr[:, di, :], xgT[:, di, :])
            raw_ps = rps.tile([E, C], f32)
            for k in range(8):
                off = 0
                for w in _mchunks(C):
                    nc.tensor.matmul(
                        raw_ps[:, off:off + w],
                        lhsT=centT[:, k, :],
                        rhs=xgT[:, k, off:off + w],
                        start=(k == 0),
                        stop=(k == 7),
                    )
                    off += w
            raw_sb = rpool.tile([E, C], f32, tag="rawsb")
            nc.vector.tensor_copy(raw_sb[:], raw_ps[:])

            for j in range(CJ):
                t_ps = tps.tile([128, E], f32)
                nc.tensor.transpose(
                    t_ps[:], raw_sb[:, j * 128:(j + 1) * 128], ident[0:E, 0:E]
                )
                raw_t = rpool.tile([128, E], f32, tag="rawt")
                nc.vector.tensor_copy(raw_t[:], t_ps[:])
                braw = rpool.tile([128, E], f32, tag="braw")
                nc.vector.tensor_add(braw[:], raw_t[:], rbias[:])
                scores = rpool.tile([128, E], f32, tag="scores")
                nc.scalar.activation(scores[:], raw_t[:], AF.Sigmoid, bias=zbias[:])
                maxes = rpool.tile([128, 8], f32, tag="maxes")
                nc.vector.max(maxes[:], braw[:])
                mask1 = rpool.tile([128, E], f32, tag="mask1")
                nc.vector.tensor_tensor(
                    mask1[:], braw[:], maxes[:, 0:1].to_broadcast([128, E]), ALU.is_ge
                )
                mask2 = rpool.tile([128, E], f32, tag="mask2")
                nc.vector.tensor_tensor(
                    mask2[:], braw[:], maxes[:, 1:2].to_broadcast([128, E]), ALU.is_ge
                )
                nc.vector.tensor_sub(mask2[:], mask2[:], mask1[:])
                tmp = rpool.tile([128, E], f32, tag="tmp")
                s1 = rpool.tile([128, 1], f32, tag="s1")
                s2 = rpool.tile([128, 1], f32, tag="s2")
                nc.vector.tensor_mul(tmp[:], mask1[:], scores[:])
                nc.vector.tensor_reduce(s1[:], tmp[:], axis=AX.X, op=ALU.add)
                nc.vector.tensor_mul(tmp[:], mask2[:], scores[:])
                nc.vector.tensor_reduce(s2[:], tmp[:], axis=AX.X, op=ALU.add)
                i1 = rpool.tile([128, 1], f32, tag="i1")
                i2 = rpool.tile([128, 1], f32, tag="i2")
                nc.vector.tensor_mul(tmp[:], mask1[:], sel[:])
                nc.vector.tensor_reduce(i1[:], tmp[:], axis=AX.X, op=ALU.add)
                nc.vector.tensor_mul(tmp[:], mask2[:], sel[:])
                nc.vector.tensor_reduce(i2[:], tmp[:], axis=AX.X, op=ALU.add)
                w1 = rpool.tile([128, 1], f32, tag="w1")
                nc.vector.tensor_sub(s1[:], s1[:], s2[:])
                nc.scalar.activation(w1[:], s1[:], AF.Sigmoid, bias=zbias[:])
                w2 = rpool.tile([128, 1], f32, tag="w2")
                nc.vector.tensor_scalar(
                    w2[:], w1[:], -1.0, 1.0, op0=ALU.mult, op1=ALU.add
                )
                nc.vector.tensor_mul(i1[:], i1[:], w1[:])
                nc.vector.tensor_mul(i2[:], i2[:], w2[:])
                nc.vector.tensor_add(i1[:], i1[:], i2[:])
                nc.vector.tensor_mul(ce[:, j:j + 1], i1[:], valid[:, j:j + 1])

        # ---- MLP passes -----------------------------------------------------
        with tc.tile_pool(name="ph", bufs=2, space="PSUM") as php, \
             tc.tile_pool(name="py", bufs=2, space="PSUM") as pyp:

            def mlp(xsb, d_wfc_, d_wpj_, yacc, tcnt, fh):
                njt = tcnt // 128
                nch = fh // FCH
                mt = FCH // 128
                for ci in range(nch):
                    wfc_t = wpool.tile([128, 8, FCH], f32r, tag="wfc")
                    for di in range(8):
                        nc.sync.dma_start(
                            wfc_t[:, di, :],
                            d_wfc_[di * 128:(di + 1) * 128,
                                   ci * FCH:(ci + 1) * FCH],
                        )
                    wpj_t = wpool.tile([128, mt, D], f32r, tag="wpj")
                    for mi in range(mt):
                        nc.sync.dma_start(
                            wpj_t[:, mi, :],
                            d_wpj_[ci * FCH + mi * 128:ci * FCH + (mi + 1) * 128, :],
                        )
                    hs = []
                    for mi in range(mt):
                        ph = php.tile([128, C], f32, tag="ph")
                        for k in range(8):
                            off = 0
                            for w in _mchunks(tcnt):
                                nc.tensor.matmul(
                                    ph[:, off:off + w],
                                    lhsT=wfc_t[:, k, mi * 128:(mi + 1) * 128],
                                    rhs=xsb[:, k, off:off + w],
                                    start=(k == 0),
                                    stop=(k == 7),
                                )
                                off += w
                        h = hpool.tile([128, C], f32r, tag=f"h{mi}")
                        nc.scalar.activation(
                            h[:, 0:tcnt], ph[:, 0:tcnt], AF.Gelu, bias=zbias[:]
                        )
                        hs.append(h)
                    for j in range(njt):
                        py = pyp.tile([128, D], f32, tag="py")
                        for mi in range(mt):
                            off = 0
                            for w in _mchunks(D):
                                nc.tensor.matmul(
                                    py[:, off:off + w],
                                    lhsT=hs[mi][:, j * 128:(j + 1) * 128],
                                    rhs=wpj_t[:, mi, off:off + w],
                                    start=(mi == 0),
                                    stop=(mi == mt - 1),
                                )
                                off += w
                        if ci == 0:
                            nc.vector.tensor_copy(yacc[:, j, :], py[:])
                        else:
                            nc.vector.tensor_add(yacc[:, j, :], yacc[:, j, :], py[:])

            mlp(xgTr, d_wfc, d_wpj, y_r, C, F)
            mlp(xqT, d_wfcs, d_wpjs, y_sh, TQ, FSH)

        # ---- finalize + store ----------------------------------------------
        for j in range(CJ):
            nc.vector.tensor_scalar_mul(y_r[:, j, :], y_r[:, j, :], ce[:, j:j + 1])
            nc.sync.dma_start(d_outr[j * 128:(j + 1) * 128, :], y_r[:, j, :])
        for j in range(QJ):
            nc.vector.tensor_add(y_sh[:, j, :], y_sh[:, j, :], xres[:, j, :])
            nc.sync.dma_start(d_outs[j * 128:(j + 1) * 128, :], y_sh[:, j, :])

    if not nc.is_finalized():
        nc.finalize()
    return nc


def kernel(x, centroids, routing_bias, Wfc_r, Wproj_r, Wfc_sh, Wproj_sh):
    from concourse import bass_utils

    x = np.asarray(x, np.float32)
    centroids = np.asarray(centroids, np.float32)
    routing_bias = np.asarray(routing_bias, np.float32)
    Wfc_r = np.asarray(Wfc_r, np.float32)
    Wproj_r = np.asarray(Wproj_r, np.float32)
    Wfc_sh = np.asarray(Wfc_sh, np.float32)
    Wproj_sh = np.asarray(Wproj_sh, np.float32)

    xf = np.ascontiguousarray(x.reshape(TT, D))

    # host routing — data placement only (device recomputes combine weights)
    raw = xf @ centroids.T
    balanced = raw + routing_bias[None, :]
    top2 = np.argsort(-balanced, axis=-1, kind="stable")[:, :2]
    idx_lists = []
    for e in range(E):
        hit = (top2 == e).any(axis=1)
        idx_lists.append(np.nonzero(hit)[0].astype(np.int64))
    nmax = max(len(ix) for ix in idx_lists)
    C = max(256, ((nmax + 127) // 128) * 128)

    if C not in _prog_cache:
        _prog_cache[C] = _build(C)
    nc = _prog_cache[C]

    xT = np.ascontiguousarray(xf.T)
    ident = np.eye(128, dtype=np.float32)
    rbias128 = np.tile(routing_bias[None, :], (128, 1)).astype(np.float32)
    FH = F // 2  # shared expert F-half

    in_maps = []
    for c in range(E):
        ix = idx_lists[c]
        n = len(ix)
        pad = np.zeros(C, np.int64)
        pad[:n] = ix
        xg = xf[pad]  # [C, D]
        validm = np.zeros((128, C // 128), np.float32)
        for t in range(C):
            if t < n:
                validm[t % 128, t // 128] = 1.0
        sel = np.zeros((128, E), np.float32)
        sel[:, c] = 1.0
        q = c // 2
        half = c % 2
        wfcsh = np.concatenate(
            [Wfc_sh[nn, half * FH:(half + 1) * FH, :] for nn in range(NS)], axis=0
        )  # [FSH, D]
        wpjsh = np.concatenate(
            [Wproj_sh[nn, :, half * FH:(half + 1) * FH] for nn in range(NS)], axis=1
        )  # [D, FSH]
        in_maps.append({
            "xgT": np.ascontiguousarray(xg.T),
            "xqT": np.ascontiguousarray(xT[:, q * TQ:(q + 1) * TQ]),
            "x_res": np.ascontiguousarray(xf[q * TQ:(q + 1) * TQ] * 0.5),
            "centT": np.ascontiguousarray(centroids.T),
            "rbias": rbias128,
            "sel": sel,
            "valid": validm,
            "ident": ident,
            "wfcT": np.ascontiguousarray(Wfc_r[c].T),
            "wprojT": np.ascontiguousarray(Wproj_r[c].T),
            "wfcshT": np.ascontiguousarray(wfcsh.T),
            "wprojshT": np.ascontiguousarray(wpjsh.T),
        })

    globals()["_last_in_maps"] = in_maps
    res = bass_utils.run_bass_kernel_spmd(nc, in_maps, core_ids=list(range(E)))
    globals()["_last_results"] = res

    out = np.zeros((TT, D), np.float32)
    for c in range(E):
        r = res.results[c]
        q = c // 2
        out[q * TQ:(q + 1) * TQ] += r["out_sh"]
        ix = idx_lists[c]
        out[ix] += r["out_r"][:len(ix)]
    return out.reshape(B, S, D)



# revision 20
# speedup vs baseline: 187.3855x; 187.3855x over previous
"""MoE kernel for Trainium2 — 8-core balanced half-expert sharding, bf16.

Strategy:
  - Host computes routing (replica of reference math, fp32) ONLY to decide
    data placement. Combine weights used in the output math are recomputed
    ON DEVICE from raw inputs (centroid matmul + sigmoid/top2/softmax).
  - Each routed expert is split into two F-halves -> 16 weight blocks of
    [F/2=2048, D]. The 16 blocks (each with that expert's token list) are
    ranked by token count; the 8 largest become "A" slots, the 8 smallest
    "B" slots; core c gets (A[c], B[c]). Static capacities CA=max|A|,
    CB=max|B| are ~load-balanced (CA+CB ~ 1050 vs 1152 for plain
    expert-parallel on this input).
  - Each core also computes a (token-quarter x F-half) shard of the 2
    shared experts, plus the 0.5*x residual (two cores per quarter).
  - MLP matmuls in bf16 (f32 PSUM accumulate); weights/x pre-packed on
    host into SBUF layout so every weight DMA is one contiguous-per-
    partition ~1MB transfer.
  - Host unshard: scatter-add routed partials, sum shared partials.
  - _build(CA, CB, loop_n): loop_n > 1 wraps the whole body in a hardware
    For_i loop (used by test.py to time per-iteration HW cost with the
    dispatch overhead amortized away).
"""

import sys

sys.path.insert(0, "/opt/trn_rl_repo")

import numpy as np
import ml_dtypes

BF16 = ml_dtypes.bfloat16

D = 1024
F = 4096          # routed expert hidden
FHALF = F // 2    # routed block hidden (expert F-half)
FSH = 4096        # shared shard hidden: 2 experts x (4096/2) F-half
E = 8
NS = 2
B, S = 2, 1024
TT = B * S        # 2048 tokens
TQ = TT // 4      # 512-token quarter per shared shard
FCH = 512         # hidden-chunk streamed per iteration

_prog_cache = {}


def _mchunks(n):
    """Split n into moving-dim chunks of 512 + remainder. Chunks must start
    at multiples of 512 so no matmul output crosses a PSUM bank boundary
    (fp32 bank = 512 floats)."""
    out = [512] * (n // 512)
    if n % 512:
        out.append(n % 512)
    return out


def _jtiles(n):
    """(offset, width) 128-row output tiles covering n tokens."""
    out = []
    o = 0
    while o < n:
        out.append((o, min(128, n - o)))
        o += 128
    return out


def _build(CA, CB, loop_n=1):
    from contextlib import ExitStack
    from concourse import bacc, bass, tile, mybir

    f32 = mybir.dt.float32
    bf16 = mybir.dt.bfloat16
    AF = mybir.ActivationFunctionType
    ALU = mybir.AluOpType
    AX = mybir.AxisListType
    ET = mybir.EngineType

    nc = bacc.Bacc("TRN2", target_bir_lowering=False, debug=False, num_devices=8)

    NCH_R = FHALF // FCH   # 4 chunks per routed block
    NCH_S = FSH // FCH     # 8 chunks for shared shard

    JA = _jtiles(CA)
    JB = _jtiles(CB)
    JQ = _jtiles(TQ)

    d_xa = nc.dram_tensor("xa", [128, 8 * CA], bf16, kind="ExternalInput").ap()
    d_xb = nc.dram_tensor("xb", [128, 8 * CB], bf16, kind="ExternalInput").ap()
    d_xq = nc.dram_tensor("xq", [128, 8 * TQ], bf16, kind="ExternalInput").ap()
    d_xres = nc.dram_tensor("x_res", [TQ, D], f32, kind="ExternalInput").ap()
    d_cent = nc.dram_tensor("centT", [128, 8 * E], bf16, kind="ExternalInput").ap()
    # host-computed top-2 rank masks (dispatch metadata): m1/m2 pick the
    # top-1/top-2 expert column per token; r1 is 1.0 where this slot's
    # expert is the token's top-1.
    d_m1a = nc.dram_tensor("m1a", [128, len(JA) * E], f32, kind="ExternalInput").ap()
    d_m2a = nc.dram_tensor("m2a", [128, len(JA) * E], f32, kind="ExternalInput").ap()
    d_r1a = nc.dram_tensor("r1a", [128, len(JA)], f32, kind="ExternalInput").ap()
    d_m1b = nc.dram_tensor("m1b", [128, len(JB) * E], f32, kind="ExternalInput").ap()
    d_m2b = nc.dram_tensor("m2b", [128, len(JB) * E], f32, kind="ExternalInput").ap()
    d_r1b = nc.dram_tensor("r1b", [128, len(JB)], f32, kind="ExternalInput").ap()
    d_ident = nc.dram_tensor("ident", [128, 128], f32, kind="ExternalInput").ap()
    d_wfcA = nc.dram_tensor("wfcA", [128, NCH_R * 4096], bf16, kind="ExternalInput").ap()
    d_wpjA = nc.dram_tensor("wpjA", [128, NCH_R * 4096], bf16, kind="ExternalInput").ap()
    d_wfcB = nc.dram_tensor("wfcB", [128, NCH_R * 4096], bf16, kind="ExternalInput").ap()
    d_wpjB = nc.dram_tensor("wpjB", [128, NCH_R * 4096], bf16, kind="ExternalInput").ap()
    d_wfcS = nc.dram_tensor("wfcS", [128, NCH_S * 4096], bf16, kind="ExternalInput").ap()
    d_wpjS = nc.dram_tensor("wpjS", [128, NCH_S * 4096], bf16, kind="ExternalInput").ap()
    d_outA = nc.dram_tensor("outA", [CA, D], f32, kind="ExternalOutput").ap()
    d_outB = nc.dram_tensor("outB", [CB, D], f32, kind="ExternalOutput").ap()
    d_outS = nc.dram_tensor("outS", [TQ, D], f32, kind="ExternalOutput").ap()

    with tile.TileContext(nc) as tc, ExitStack() as ctx:
        const = ctx.enter_context(tc.tile_pool(name="const", bufs=1))
        xpool = ctx.enter_context(tc.tile_pool(name="xpool", bufs=1))
        ypool = ctx.enter_context(tc.tile_pool(name="ypool", bufs=1))
        rpool = ctx.enter_context(tc.tile_pool(name="rpool", bufs=2))
        wfp = ctx.enter_context(tc.tile_pool(name="wfp", bufs=2))
        wpp = ctx.enter_context(tc.tile_pool(name="wpp", bufs=2))
        hpool = ctx.enter_context(tc.tile_pool(name="hpool", bufs=2))
        php = ctx.enter_context(tc.tile_pool(name="php", bufs=2, space="PSUM"))
        pyp = ctx.enter_context(tc.tile_pool(name="pyp", bufs=2, space="PSUM"))
        rps = ctx.enter_context(tc.tile_pool(name="rps", bufs=1, space="PSUM"))

        def body():
            # ---- resident loads (ACT HWDGE ring; weights go on SP's) ----
            cent = const.tile([128, 8 * E], bf16, tag="cent")
            nc.scalar.dma_start(cent[:], d_cent[:, :])
            ident = const.tile([128, 128], f32, tag="ident")
            nc.scalar.dma_start(ident[:], d_ident[:, :])
            m1a = const.tile([128, len(JA) * E], f32, tag="m1a")
            nc.scalar.dma_start(m1a[:], d_m1a[:, :])
            m2a = const.tile([128, len(JA) * E], f32, tag="m2a")
            nc.scalar.dma_start(m2a[:], d_m2a[:, :])
            r1a = const.tile([128, len(JA)], f32, tag="r1a")
            nc.scalar.dma_start(r1a[:], d_r1a[:, :])
            m1b = const.tile([128, len(JB) * E], f32, tag="m1b")
            nc.scalar.dma_start(m1b[:], d_m1b[:, :])
            m2b = const.tile([128, len(JB) * E], f32, tag="m2b")
            nc.scalar.dma_start(m2b[:], d_m2b[:, :])
            r1b = const.tile([128, len(JB)], f32, tag="r1b")
            nc.scalar.dma_start(r1b[:], d_r1b[:, :])
            zbias = const.tile([128, 1], f32, tag="zbias")
            nc.vector.memset(zbias[:], 0.0)

            xq = xpool.tile([128, 8 * TQ], bf16, tag="xq")
            nc.scalar.dma_start(xq[:], d_xq[:, :])
            xa = xpool.tile([128, 8 * CA], bf16, tag="xa")
            nc.scalar.dma_start(xa[:], d_xa[:, :])
            xb = xpool.tile([128, 8 * CB], bf16, tag="xb")
            nc.scalar.dma_start(xb[:], d_xb[:, :])
            xres = xpool.tile([128, len(JQ), D], f32, tag="xres")
            for j, (jo, jw) in enumerate(JQ):
                nc.scalar.dma_start(xres[0:jw, j, :], d_xres[jo:jo + jw, :])

            y_a = ypool.tile([128, len(JA), D], f32, tag="ya")
            y_b = ypool.tile([128, len(JB), D], f32, tag="yb")
            y_s = ypool.tile([128, len(JQ), D], f32, tag="ys")
            ce_a = const.tile([128, len(JA)], f32, tag="cea")
            ce_b = const.tile([128, len(JB)], f32, tag="ceb")

            # ---- routing: combine-weight values on device --------------
            # (top-2 identity comes from host masks; scores/softmax here)
            def routing(xsb, C, jt, m1, m2, r1, ce):
                # raw centroid scores for this slot's tokens, in <=512
                # pieces (one PSUM bank each)
                raws = []
                for off, w in zip(
                    [sum(_mchunks(C)[:i]) for i in range(len(_mchunks(C)))],
                    _mchunks(C),
                ):
                    rp = rps.tile([E, 512], f32, tag="rawps")
                    for k in range(8):
                        nc.tensor.matmul(
                            rp[:, 0:w],
                            lhsT=cent[:, k * E:(k + 1) * E],
                            rhs=xsb[:, k * C + off:k * C + off + w],
                            start=(k == 0),
                            stop=(k == 7),
                        )
                    rsb = rpool.tile([E, 512], f32, tag="rawsb")
                    nc.vector.tensor_copy(rsb[:, 0:w], rp[:, 0:w])
                    raws.append((off, w, rsb))

                def raw_slice(jo, jw):
                    for off, w, rsb in raws:
                        if off <= jo and jo + jw <= off + w:
                            return rsb[:, jo - off:jo - off + jw]
                    raise AssertionError("j-tile crosses raw piece")

                for j, (jo, jw) in enumerate(jt):
                    t_ps = rps.tile([128, E], f32, tag="tps")
                    nc.tensor.transpose(
                        t_ps[0:jw, :], raw_slice(jo, jw), ident[0:E, 0:E]
                    )
                    scores = rpool.tile([128, E], f32, tag="scores")
                    nc.scalar.activation(
                        scores[0:jw, :], t_ps[0:jw, :], AF.Sigmoid, bias=zbias[0:jw, :]
                    )
                    tmp = rpool.tile([128, E], f32, tag="tmp")
                    s1 = rpool.tile([128, 1], f32, tag="s1")
                    s2 = rpool.tile([128, 1], f32, tag="s2")
                    nc.vector.tensor_mul(
                        tmp[0:jw, :], m1[0:jw, j * E:(j + 1) * E], scores[0:jw, :]
                    )
                    nc.vector.tensor_reduce(s1[0:jw, :], tmp[0:jw, :], axis=AX.X, op=ALU.add)
                    nc.vector.tensor_mul(
                        tmp[0:jw, :], m2[0:jw, j * E:(j + 1) * E], scores[0:jw, :]
                    )
                    nc.vector.tensor_reduce(s2[0:jw, :], tmp[0:jw, :], axis=AX.X, op=ALU.add)
                    w1 = rpool.tile([128, 1], f32, tag="w1")
                    nc.vector.tensor_sub(s1[0:jw, :], s1[0:jw, :], s2[0:jw, :])
                    nc.scalar.activation(
                        w1[0:jw, :], s1[0:jw, :], AF.Sigmoid, bias=zbias[0:jw, :]
                    )
                    # ce = w2 + r1*(w1 - w2), w2 = 1 - w1  ->  ce = (1-w1) + r1*(2*w1-1)
                    w2 = rpool.tile([128, 1], f32, tag="w2")
                    nc.vector.tensor_scalar(
                        w2[0:jw, :], w1[0:jw, :], -1.0, 1.0, op0=ALU.mult, op1=ALU.add
                    )
                    d12 = rpool.tile([128, 1], f32, tag="d12")
                    nc.vector.tensor_sub(d12[0:jw, :], w1[0:jw, :], w2[0:jw, :])
                    nc.vector.tensor_mul(d12[0:jw, :], d12[0:jw, :], r1[0:jw, j:j + 1])
                    nc.vector.tensor_add(ce[0:jw, j:j + 1], w2[0:jw, :], d12[0:jw, :])

            # ---- MLP (fc -> gelu -> proj, accumulated over F chunks) ----
            def mlp(xsb, d_wfc_, d_wpj_, yacc, C, jt, nch):
                mcs = _mchunks(C)
                moffs = [sum(mcs[:i]) for i in range(len(mcs))]
                for ci in range(nch):
                    wfc_t = wfp.tile([128, 4096], bf16, tag="wfc")
                    nc.sync.dma_start(
                        wfc_t[:], d_wfc_[:, ci * 4096:(ci + 1) * 4096]
                    )
                    wpj_t = wpp.tile([128, 4096], bf16, tag="wpj")
                    nc.sync.dma_start(
                        wpj_t[:], d_wpj_[:, ci * 4096:(ci + 1) * 4096]
                    )
                    hs = []
                    for mi in range(4):
                        ph = php.tile([128, CA], f32, tag="ph")
                        for k in range(8):
                            for off, w in zip(moffs, mcs):
                                nc.tensor.matmul(
                                    ph[:, off:off + w],
                                    lhsT=wfc_t[:, k * 512 + mi * 128:k * 512 + (mi + 1) * 128],
                                    rhs=xsb[:, k * C + off:k * C + off + w],
                                    start=(k == 0),
                                    stop=(k == 7),
                                )
                        h = hpool.tile([128, CA], bf16, tag=f"h{mi}")
                        nc.scalar.activation(
                            h[:, 0:C], ph[:, 0:C], AF.Gelu, bias=zbias[:]
                        )
                        hs.append(h)
                    for j, (jo, jw) in enumerate(jt):
                        for dh in range(2):
                            py = pyp.tile([128, 512], f32, tag="py")
                            for mi in range(4):
                                nc.tensor.matmul(
                                    py[0:jw, :],
                                    lhsT=hs[mi][:, jo:jo + jw],
                                    rhs=wpj_t[:, mi * 1024 + dh * 512:mi * 1024 + (dh + 1) * 512],
                                    start=(mi == 0),
                                    stop=(mi == 3),
                                )
                            ys = yacc[0:jw, j, dh * 512:(dh + 1) * 512]
                            if ci == 0:
                                nc.vector.tensor_copy(ys, py[0:jw, :])
                            else:
                                nc.vector.tensor_add(ys, ys, py[0:jw, :])

            mlp(xq, d_wfcS, d_wpjS, y_s, TQ, JQ, NCH_S)
            routing(xa, CA, JA, m1a, m2a, r1a, ce_a)
            routing(xb, CB, JB, m1b, m2b, r1b, ce_b)
            mlp(xa, d_wfcA, d_wpjA, y_a, CA, JA, NCH_R)
            mlp(xb, d_wfcB, d_wpjB, y_b, CB, JB, NCH_R)

            # ---- finalize + store (stores on the SWDGE/gpsimd queue) ----
            for j, (jo, jw) in enumerate(JQ):
                nc.vector.tensor_add(y_s[0:jw, j, :], y_s[0:jw, j, :], xres[0:jw, j, :])
                nc.gpsimd.dma_start(d_outS[jo:jo + jw, :], y_s[0:jw, j, :])
            for j, (jo, jw) in enumerate(JA):
                nc.vector.tensor_scalar_mul(y_a[0:jw, j, :], y_a[0:jw, j, :], ce_a[0:jw, j:j + 1])
                nc.gpsimd.dma_start(d_outA[jo:jo + jw, :], y_a[0:jw, j, :])
            for j, (jo, jw) in enumerate(JB):
                nc.vector.tensor_scalar_mul(y_b[0:jw, j, :], y_b[0:jw, j, :], ce_b[0:jw, j:j + 1])
                nc.gpsimd.dma_start(d_outB[jo:jo + jw, :], y_b[0:jw, j, :])

        if loop_n > 1:
            with tc.For_i(
                0, loop_n, 1,
                hint_engines=(ET.PE, ET.DVE, ET.Activation, ET.SP, ET.Pool),
                name="reps",
            ):
                body()
        else:
            body()

    if not nc.is_finalized():
        nc.finalize()
    return nc


# ---- host-side packing helpers ------------------------------------------

def _pack_xT(xT, C, dtype=BF16):
    """xT [D, n<=C] (contraction-major) -> [128, 8*C] SBUF layout,
    zero-padded to C columns. tile[p, k*C+c] = xT[k*128+p, c]."""
    n = xT.shape[1]
    out = np.zeros((8, 128, C), np.float32)
    out[:, :, :n] = xT.reshape(8, 128, n)
    return np.ascontiguousarray(out.transpose(1, 0, 2).reshape(128, 8 * C)).astype(dtype)


def _pack_lhsT(WT):
    """WT [D=1024, Fdim] -> [128, (Fdim/512)*4096] bf16.
    chunk ci cols [ci*4096:(ci+1)*4096] hold tile[p, k*512+c] = WT[k*128+p, ci*512+c]."""
    Dd, Fdim = WT.shape
    nch = Fdim // FCH
    t = WT.reshape(8, 128, nch, FCH).transpose(1, 2, 0, 3)  # [p, ci, k, c]
    return np.ascontiguousarray(t.reshape(128, nch * 8 * FCH)).astype(BF16)


def _pack_rhs(WT):
    """WT [Fdim, D=1024] (proj, F-major) -> [128, (Fdim/512)*4096] bf16.
    chunk ci holds tile[p, mi*1024+d] = WT[ci*512+mi*128+p, d]."""
    Fdim, Dd = WT.shape
    nch = Fdim // FCH
    t = WT.reshape(nch, 4, 128, Dd).transpose(2, 0, 1, 3)  # [p, ci, mi, d]
    return np.ascontiguousarray(t.reshape(128, nch * 4 * Dd)).astype(BF16)


def _roundup(n, m):
    return ((n + m - 1) // m) * m


def _prepare(x, centroids, routing_bias, Wfc_r, Wproj_r, Wfc_sh, Wproj_sh):
    """Host-side routing/dispatch: returns (in_maps, meta)."""
    x = np.asarray(x, np.float32)
    centroids = np.asarray(centroids, np.float32)
    routing_bias = np.asarray(routing_bias, np.float32)
    Wfc_r = np.asarray(Wfc_r, np.float32)
    Wproj_r = np.asarray(Wproj_r, np.float32)
    Wfc_sh = np.asarray(Wfc_sh, np.float32)
    Wproj_sh = np.asarray(Wproj_sh, np.float32)

    xf = np.ascontiguousarray(x.reshape(TT, D))

    # host routing — data placement only (device recomputes combine weights)
    raw = xf @ centroids.T
    balanced = raw + routing_bias[None, :]
    top2 = np.argsort(-balanced, axis=-1, kind="stable")[:, :2]
    idx_lists = []
    for e in range(E):
        hit = (top2 == e).any(axis=1)
        idx_lists.append(np.nonzero(hit)[0].astype(np.int64))

    # 16 blocks = (expert, F-half); rank by token count, big 8 are A slots
    blocks = [(e, h) for e in range(E) for h in range(2)]
    blocks.sort(key=lambda b: -len(idx_lists[b[0]]))
    A_blocks = blocks[:8]
    B_blocks = blocks[8:]
    CA = max(128, _roundup(max(len(idx_lists[e]) for e, _ in A_blocks), 4))
    CB = max(128, _roundup(max(len(idx_lists[e]) for e, _ in B_blocks), 4))

    xT = np.ascontiguousarray(xf.T)
    ident = np.eye(128, dtype=np.float32)
    centT_packed = _pack_xT(np.ascontiguousarray(centroids.T), E)
    FH2 = F // 2

    def rank_masks(ix, C, expert):
        """Host dispatch metadata: top-1/top-2 one-hot masks + is-top-1
        flag for this slot's tokens, laid out [row=tok%128, tile=tok//128]."""
        nt = len(_jtiles(C))
        m1 = np.zeros((128, nt, E), np.float32)
        m2 = np.zeros((128, nt, E), np.float32)
        r1 = np.zeros((128, nt), np.float32)
        for p, t in enumerate(ix):
            row, tile = p % 128, p // 128
            m1[row, tile, top2[t, 0]] = 1.0
            m2[row, tile, top2[t, 1]] = 1.0
            if top2[t, 0] == expert:
                r1[row, tile] = 1.0
        return (
            np.ascontiguousarray(m1.reshape(128, nt * E)),
            np.ascontiguousarray(m2.reshape(128, nt * E)),
            r1,
        )

    in_maps = []
    for c in range(E):
        eA, hA = A_blocks[c]
        eB, hB = B_blocks[c]
        ixA = idx_lists[eA]
        ixB = idx_lists[eB]
        m1a, m2a, r1a = rank_masks(ixA, CA, eA)
        m1b, m2b, r1b = rank_masks(ixB, CB, eB)
        q = c // 2
        half = c % 2
        wfcsh = np.concatenate(
            [Wfc_sh[n, half * FH2:(half + 1) * FH2, :] for n in range(NS)], axis=0
        )  # [FSH, D]
        wpjsh = np.concatenate(
            [Wproj_sh[n, :, half * FH2:(half + 1) * FH2] for n in range(NS)], axis=1
        )  # [D, FSH]
        in_maps.append({
            "xa": _pack_xT(xT[:, ixA], CA),
            "xb": _pack_xT(xT[:, ixB], CB),
            "xq": _pack_xT(xT[:, q * TQ:(q + 1) * TQ], TQ),
            "x_res": np.ascontiguousarray(xf[q * TQ:(q + 1) * TQ] * 0.5),
            "centT": centT_packed,
            "m1a": m1a, "m2a": m2a, "r1a": r1a,
            "m1b": m1b, "m2b": m2b, "r1b": r1b,
            "ident": ident,
            "wfcA": _pack_lhsT(Wfc_r[eA, hA * FHALF:(hA + 1) * FHALF, :].T),
            "wpjA": _pack_rhs(Wproj_r[eA, :, hA * FHALF:(hA + 1) * FHALF].T),
            "wfcB": _pack_lhsT(Wfc_r[eB, hB * FHALF:(hB + 1) * FHALF, :].T),
            "wpjB": _pack_rhs(Wproj_r[eB, :, hB * FHALF:(hB + 1) * FHALF].T),
            "wfcS": _pack_lhsT(np.ascontiguousarray(wfcsh.T)),
            "wpjS": _pack_rhs(np.ascontiguousarray(wpjsh.T)),
        })
    meta = (A_blocks, B_blocks, CA, CB, idx_lists)
    return in_maps, meta


def _assemble(results, meta):
    A_blocks, B_blocks, CA, CB, idx_lists = meta
    out = np.zeros((TT, D), np.float32)
    for c in range(E):
        r = results[c]
        q = c // 2
        out[q * TQ:(q + 1) * TQ] += r["outS"]
        eA, _ = A_blocks[c]
        eB, _ = B_blocks[c]
        ixA = idx_lists[eA]
        ixB = idx_lists[eB]
        out[ixA] += np.asarray(r["outA"])[:len(ixA)]
        out[ixB] += np.asarray(r["outB"])[:len(ixB)]
    return out.reshape(B, S, D)


def kernel(x, centroids, routing_bias, Wfc_r, Wproj_r, Wfc_sh, Wproj_sh):
    from concourse import bass_utils

    in_maps, meta = _prepare(
        x, centroids, routing_bias, Wfc_r, Wproj_r, Wfc_sh, Wproj_sh
    )
    CA, CB = meta[2], meta[3]
    if (CA, CB) not in _prog_cache:
        _prog_cache[(CA, CB)] = _build(CA, CB)
    nc = _prog_cache[(CA, CB)]

    globals()["_last_in_maps"] = in_maps
    globals()["_last_meta"] = meta
    res = bass_utils.run_bass_kernel_spmd(nc, in_maps, core_ids=list(range(E)))
    globals()["_last_results"] = res
    return _assemble(res.results, meta)


# revision 30
# speedup vs baseline: 188.0443x; 1.0035x over previous
"""MoE kernel for Trainium2 — 8-core balanced half-expert sharding, bf16.

Strategy:
  - Host computes routing (replica of reference math, fp32) ONLY to decide
    data placement. Combine weights used in the output math are recomputed
    ON DEVICE from raw inputs (centroid matmul + sigmoid/top2/softmax).
  - Each routed expert is split into two F-halves -> 16 weight blocks of
    [F/2=2048, D]. The 16 blocks (each with that expert's token list) are
    ranked by token count; the 8 largest become "A" slots, the 8 smallest
    "B" slots; core c gets (A[c], B[c]). Static capacities CA=max|A|,
    CB=max|B| are ~load-balanced (CA+CB ~ 1050 vs 1152 for plain
    expert-parallel on this input).
  - Each core also computes a (token-quarter x F-half) shard of the 2
    shared experts, plus the 0.5*x residual (two cores per quarter).
  - MLP matmuls in bf16 (f32 PSUM accumulate); weights/x pre-packed on
    host into SBUF layout so every weight DMA is one contiguous-per-
    partition ~1MB transfer.
  - Host unshard: scatter-add routed partials, sum shared partials.
  - _build(CA, CB, loop_n): loop_n > 1 wraps the whole body in a hardware
    For_i loop (used by test.py to time per-iteration HW cost with the
    dispatch overhead amortized away).
"""

import sys

sys.path.insert(0, "/opt/trn_rl_repo")

import numpy as np
import ml_dtypes

BF16 = ml_dtypes.bfloat16

D = 1024
F = 4096          # routed expert hidden
FHALF = F // 2    # routed block hidden (expert F-half)
FSH = 4096        # shared shard hidden: 2 experts x (4096/2) F-half
E = 8
NS = 2
B, S = 2, 1024
TT = B * S        # 2048 tokens
TQ = TT // 4      # 512-token quarter per shared shard
FCH = 512         # hidden-chunk streamed per iteration

_prog_cache = {}


def _mchunks(n):
    """Split n into moving-dim chunks of 512 + remainder. Chunks must start
    at multiples of 512 so no matmul output crosses a PSUM bank boundary
    (fp32 bank = 512 floats)."""
    out = [512] * (n // 512)
    if n % 512:
        out.append(n % 512)
    return out


def _jtiles(n):
    """(offset, width) 128-row output tiles covering n tokens."""
    out = []
    o = 0
    while o < n:
        out.append((o, min(128, n - o)))
        o += 128
    return out


def _build(CA, CB, loop_n=1, parts="all"):
    """parts: 'all' = the real kernel; microbench variants: 'dma' = only the
    HBM loads, 'pe' = compute with weights resident (no per-chunk weight
    DMA), 'loop' = empty body (For_i back-edge cost)."""
    from contextlib import ExitStack
    from concourse import bacc, bass, tile, mybir

    f32 = mybir.dt.float32
    bf16 = mybir.dt.bfloat16
    AF = mybir.ActivationFunctionType
    ALU = mybir.AluOpType
    AX = mybir.AxisListType
    ET = mybir.EngineType

    nc = bacc.Bacc("TRN2", target_bir_lowering=False, debug=False, num_devices=8)

    NCH_R = FHALF // FCH   # 4 chunks per routed block
    NCH_S = FSH // FCH     # 8 chunks for shared shard

    JA = _jtiles(CA)
    JB = _jtiles(CB)
    JQ = _jtiles(TQ)

    d_xa = nc.dram_tensor("xa", [128, 8 * CA], bf16, kind="ExternalInput").ap()
    d_xb = nc.dram_tensor("xb", [128, 8 * CB], bf16, kind="ExternalInput").ap()
    d_xq = nc.dram_tensor("xq", [128, 8 * TQ], bf16, kind="ExternalInput").ap()
    d_xres = nc.dram_tensor("x_res", [TQ, D], f32, kind="ExternalInput").ap()
    d_cent = nc.dram_tensor("centT", [128, 8 * E], bf16, kind="ExternalInput").ap()
    # misc blob: ident(128) + host-computed top-2 rank dispatch masks
    # m1a/m2a(JA*E each) + r1a(JA) + m1b/m2b(JB*E) + r1b(JB)
    NMISC = 128 + 2 * len(JA) * E + len(JA) + 2 * len(JB) * E + len(JB)
    d_misc = nc.dram_tensor("misc", [128, NMISC], f32, kind="ExternalInput").ap()
    d_wfcA = nc.dram_tensor("wfcA", [128, NCH_R * 4096], bf16, kind="ExternalInput").ap()
    d_wpjA = nc.dram_tensor("wpjA", [128, NCH_R * 4096], bf16, kind="ExternalInput").ap()
    d_wfcB = nc.dram_tensor("wfcB", [128, NCH_R * 4096], bf16, kind="ExternalInput").ap()
    d_wpjB = nc.dram_tensor("wpjB", [128, NCH_R * 4096], bf16, kind="ExternalInput").ap()
    d_wfcS = nc.dram_tensor("wfcS", [128, NCH_S * 4096], bf16, kind="ExternalInput").ap()
    d_wpjS = nc.dram_tensor("wpjS", [128, NCH_S * 4096], bf16, kind="ExternalInput").ap()
    d_outA = nc.dram_tensor("outA", [CA, D], bf16, kind="ExternalOutput").ap()
    d_outB = nc.dram_tensor("outB", [CB, D], bf16, kind="ExternalOutput").ap()
    d_outS = nc.dram_tensor("outS", [TQ, D], bf16, kind="ExternalOutput").ap()

    with tile.TileContext(nc) as tc, ExitStack() as ctx:
        const = ctx.enter_context(tc.tile_pool(name="const", bufs=1))
        xpool = ctx.enter_context(tc.tile_pool(name="xpool", bufs=1))
        ypool = ctx.enter_context(tc.tile_pool(name="ypool", bufs=1))
        rpool = ctx.enter_context(tc.tile_pool(name="rpool", bufs=2))
        wfp = ctx.enter_context(tc.tile_pool(name="wfp", bufs=3))
        wpp = ctx.enter_context(tc.tile_pool(name="wpp", bufs=3))
        hpool = ctx.enter_context(tc.tile_pool(name="hpool", bufs=2))
        opool = ctx.enter_context(tc.tile_pool(name="opool", bufs=3))
        php = ctx.enter_context(tc.tile_pool(name="php", bufs=2, space="PSUM"))
        pyp = ctx.enter_context(tc.tile_pool(name="pyp", bufs=2, space="PSUM"))
        rps = ctx.enter_context(tc.tile_pool(name="rps", bufs=1, space="PSUM"))

        def body():
            # ---- resident loads (ACT HWDGE ring; weights go on SP's).
            # x tensors first: the first MLP chunk waits on xq+cent only.
            xq = xpool.tile([128, 8 * TQ], bf16, tag="xq")
            nc.scalar.dma_start(xq[:], d_xq[:, :])
            cent = const.tile([128, 8 * E], bf16, tag="cent")
            nc.scalar.dma_start(cent[:], d_cent[:, :])
            xa = xpool.tile([128, 8 * CA], bf16, tag="xa")
            nc.scalar.dma_start(xa[:], d_xa[:, :])
            xb = xpool.tile([128, 8 * CB], bf16, tag="xb")
            nc.scalar.dma_start(xb[:], d_xb[:, :])
            misc = const.tile([128, NMISC], f32, tag="misc")
            nc.scalar.dma_start(misc[:], d_misc[:, :])
            o = 0
            ident = misc[:, o:o + 128]; o += 128
            m1a = misc[:, o:o + len(JA) * E]; o += len(JA) * E
            m2a = misc[:, o:o + len(JA) * E]; o += len(JA) * E
            r1a = misc[:, o:o + len(JA)]; o += len(JA)
            m1b = misc[:, o:o + len(JB) * E]; o += len(JB) * E
            m2b = misc[:, o:o + len(JB) * E]; o += len(JB) * E
            r1b = misc[:, o:o + len(JB)]; o += len(JB)
            zbias = const.tile([128, 1], f32, tag="zbias")
            nc.vector.memset(zbias[:], 0.0)
            xres = xpool.tile([128, len(JQ), D], f32, tag="xres")
            for j, (jo, jw) in enumerate(JQ):
                nc.scalar.dma_start(xres[0:jw, j, :], d_xres[jo:jo + jw, :])

            y_a = ypool.tile([128, len(JA), D], f32, tag="ya")
            y_b = ypool.tile([128, len(JB), D], f32, tag="yb")
            y_s = ypool.tile([128, len(JQ), D], f32, tag="ys")
            ce_a = const.tile([128, len(JA)], f32, tag="cea")
            ce_b = const.tile([128, len(JB)], f32, tag="ceb")

            # ---- routing: combine-weight values on device --------------
            # (top-2 identity comes from host masks; scores/softmax here)
            def routing(xsb, C, jt, m1, m2, r1, ce):
                # raw centroid scores for this slot's tokens, in <=512
                # pieces (one PSUM bank each)
                raws = []
                for off, w in zip(
                    [sum(_mchunks(C)[:i]) for i in range(len(_mchunks(C)))],
                    _mchunks(C),
                ):
                    rp = rps.tile([E, 512], f32, tag="rawps")
                    for k in range(8):
                        nc.tensor.matmul(
                            rp[:, 0:w],
                            lhsT=cent[:, k * E:(k + 1) * E],
                            rhs=xsb[:, k * C + off:k * C + off + w],
                            start=(k == 0),
                            stop=(k == 7),
                        )
                    rsb = rpool.tile([E, 512], f32, tag="rawsb")
                    nc.vector.tensor_copy(rsb[:, 0:w], rp[:, 0:w])
                    raws.append((off, w, rsb))

                def raw_slice(jo, jw):
                    for off, w, rsb in raws:
                        if off <= jo and jo + jw <= off + w:
                            return rsb[:, jo - off:jo - off + jw]
                    raise AssertionError("j-tile crosses raw piece")

                for j, (jo, jw) in enumerate(jt):
                    t_ps = rps.tile([128, E], f32, tag="tps")
                    nc.tensor.transpose(
                        t_ps[0:jw, :], raw_slice(jo, jw), ident[0:E, 0:E]
                    )
                    scores = rpool.tile([128, E], f32, tag="scores")
                    nc.scalar.activation(
                        scores[0:jw, :], t_ps[0:jw, :], AF.Sigmoid, bias=zbias[0:jw, :]
                    )
                    tmp = rpool.tile([128, E], f32, tag="tmp")
                    s1 = rpool.tile([128, 1], f32, tag="s1")
                    s2 = rpool.tile([128, 1], f32, tag="s2")
                    nc.vector.tensor_mul(
                        tmp[0:jw, :], m1[0:jw, j * E:(j + 1) * E], scores[0:jw, :]
                    )
                    nc.vector.tensor_reduce(s1[0:jw, :], tmp[0:jw, :], axis=AX.X, op=ALU.add)
                    nc.vector.tensor_mul(
                        tmp[0:jw, :], m2[0:jw, j * E:(j + 1) * E], scores[0:jw, :]
                    )
                    nc.vector.tensor_reduce(s2[0:jw, :], tmp[0:jw, :], axis=AX.X, op=ALU.add)
                    w1 = rpool.tile([128, 1], f32, tag="w1")
                    nc.vector.tensor_sub(s1[0:jw, :], s1[0:jw, :], s2[0:jw, :])
                    nc.scalar.activation(
                        w1[0:jw, :], s1[0:jw, :], AF.Sigmoid, bias=zbias[0:jw, :]
                    )
                    # ce = w2 + r1*(w1 - w2), w2 = 1 - w1  ->  ce = (1-w1) + r1*(2*w1-1)
                    w2 = rpool.tile([128, 1], f32, tag="w2")
                    nc.vector.tensor_scalar(
                        w2[0:jw, :], w1[0:jw, :], -1.0, 1.0, op0=ALU.mult, op1=ALU.add
                    )
                    d12 = rpool.tile([128, 1], f32, tag="d12")
                    nc.vector.tensor_sub(d12[0:jw, :], w1[0:jw, :], w2[0:jw, :])
                    nc.vector.tensor_mul(d12[0:jw, :], d12[0:jw, :], r1[0:jw, j:j + 1])
                    nc.vector.tensor_add(ce[0:jw, j:j + 1], w2[0:jw, :], d12[0:jw, :])

            # ---- MLP (fc -> gelu -> proj, accumulated over F chunks) ----
            def mlp(xsb, d_wfc_, d_wpj_, yacc, C, jt, nch, wres=None):
                mcs = _mchunks(C)
                moffs = [sum(mcs[:i]) for i in range(len(mcs))]
                for ci in range(nch):
                    if wres is not None:
                        wfc_t, wpj_t = wres
                    else:
                        wfc_t = wfp.tile([128, 4096], bf16, tag="wfc")
                        nc.sync.dma_start(
                            wfc_t[:], d_wfc_[:, ci * 4096:(ci + 1) * 4096]
                        )
                        wpj_t = wpp.tile([128, 4096], bf16, tag="wpj")
                        nc.sync.dma_start(
                            wpj_t[:], d_wpj_[:, ci * 4096:(ci + 1) * 4096]
                        )
                    if parts == "dma":
                        continue
                    hs = []
                    for mi in range(4):
                        ph = php.tile([128, CA], f32, tag="ph")
                        for k in range(8):
                            for off, w in zip(moffs, mcs):
                                nc.tensor.matmul(
                                    ph[:, off:off + w],
                                    lhsT=wfc_t[:, k * 512 + mi * 128:k * 512 + (mi + 1) * 128],
                                    rhs=xsb[:, k * C + off:k * C + off + w],
                                    start=(k == 0),
                                    stop=(k == 7),
                                )
                        h = hpool.tile([128, CA], bf16, tag=f"h{mi}")
                        nc.scalar.activation(
                            h[:, 0:C], ph[:, 0:C], AF.Gelu, bias=zbias[:]
                        )
                        hs.append(h)
                    for j, (jo, jw) in enumerate(jt):
                        for dh in range(2):
                            py = pyp.tile([128, 512], f32, tag="py")
                            for mi in range(4):
                                nc.tensor.matmul(
                                    py[0:jw, :],
                                    lhsT=hs[mi][:, jo:jo + jw],
                                    rhs=wpj_t[:, mi * 1024 + dh * 512:mi * 1024 + (dh + 1) * 512],
                                    start=(mi == 0),
                                    stop=(mi == 3),
                                )
                            ys = yacc[0:jw, j, dh * 512:(dh + 1) * 512]
                            if ci == 0:
                                nc.vector.tensor_copy(ys, py[0:jw, :])
                            else:
                                nc.vector.tensor_add(ys, ys, py[0:jw, :])

            mlp(xq, d_wfcS, d_wpjS, y_s, TQ, JQ, NCH_S)
            routing(xa, CA, JA, m1a, m2a, r1a, ce_a)
            routing(xb, CB, JB, m1b, m2b, r1b, ce_b)
            mlp(xa, d_wfcA, d_wpjA, y_a, CA, JA, NCH_R)
            mlp(xb, d_wfcB, d_wpjB, y_b, CB, JB, NCH_R)

            # ---- finalize + store (bf16 out, SWDGE/gpsimd queue) --------
            for j, (jo, jw) in enumerate(JQ):
                ob = opool.tile([128, D], bf16, tag="ob")
                nc.vector.tensor_add(ob[0:jw, :], y_s[0:jw, j, :], xres[0:jw, j, :])
                nc.gpsimd.dma_start(d_outS[jo:jo + jw, :], ob[0:jw, :])
            for j, (jo, jw) in enumerate(JA):
                ob = opool.tile([128, D], bf16, tag="ob")
                nc.vector.tensor_scalar_mul(ob[0:jw, :], y_a[0:jw, j, :], ce_a[0:jw, j:j + 1])
                nc.gpsimd.dma_start(d_outA[jo:jo + jw, :], ob[0:jw, :])
            for j, (jo, jw) in enumerate(JB):
                ob = opool.tile([128, D], bf16, tag="ob")
                nc.vector.tensor_scalar_mul(ob[0:jw, :], y_b[0:jw, j, :], ce_b[0:jw, j:j + 1])
                nc.gpsimd.dma_start(d_outB[jo:jo + jw, :], ob[0:jw, :])

        if loop_n > 1:
            with tc.For_i(
                0, loop_n, 1,
                hint_engines=(ET.PE, ET.DVE, ET.Activation, ET.SP, ET.Pool),
                name="reps",
            ):
                body()
        else:
            body()

    if not nc.is_finalized():
        nc.finalize()
    return nc


# ---- host-side packing helpers ------------------------------------------

def _pack_xT(xT, C, dtype=BF16):
    """xT [D, n<=C] (contraction-major) -> [128, 8*C] SBUF layout,
    zero-padded to C columns. tile[p, k*C+c] = xT[k*128+p, c]."""
    n = xT.shape[1]
    out = np.zeros((8, 128, C), np.float32)
    out[:, :, :n] = xT.reshape(8, 128, n)
    return np.ascontiguousarray(out.transpose(1, 0, 2).reshape(128, 8 * C)).astype(dtype)


def _pack_lhsT(WT):
    """WT [D=1024, Fdim] -> [128, (Fdim/512)*4096] bf16.
    chunk ci cols [ci*4096:(ci+1)*4096] hold tile[p, k*512+c] = WT[k*128+p, ci*512+c]."""
    Dd, Fdim = WT.shape
    nch = Fdim // FCH
    t = WT.reshape(8, 128, nch, FCH).transpose(1, 2, 0, 3)  # [p, ci, k, c]
    return np.ascontiguousarray(t.reshape(128, nch * 8 * FCH)).astype(BF16)


def _pack_rhs(WT):
    """WT [Fdim, D=1024] (proj, F-major) -> [128, (Fdim/512)*4096] bf16.
    chunk ci holds tile[p, mi*1024+d] = WT[ci*512+mi*128+p, d]."""
    Fdim, Dd = WT.shape
    nch = Fdim // FCH
    t = WT.reshape(nch, 4, 128, Dd).transpose(2, 0, 1, 3)  # [p, ci, mi, d]
    return np.ascontiguousarray(t.reshape(128, nch * 4 * Dd)).astype(BF16)


def _roundup(n, m):
    return ((n + m - 1) // m) * m


def _prepare(x, centroids, routing_bias, Wfc_r, Wproj_r, Wfc_sh, Wproj_sh):
    """Host-side routing/dispatch: returns (in_maps, meta)."""
    x = np.asarray(x, np.float32)
    centroids = np.asarray(centroids, np.float32)
    routing_bias = np.asarray(routing_bias, np.float32)
    Wfc_r = np.asarray(Wfc_r, np.float32)
    Wproj_r = np.asarray(Wproj_r, np.float32)
    Wfc_sh = np.asarray(Wfc_sh, np.float32)
    Wproj_sh = np.asarray(Wproj_sh, np.float32)

    xf = np.ascontiguousarray(x.reshape(TT, D))

    # host routing — data placement only (device recomputes combine weights)
    raw = xf @ centroids.T
    balanced = raw + routing_bias[None, :]
    top2 = np.argsort(-balanced, axis=-1, kind="stable")[:, :2]
    idx_lists = []
    for e in range(E):
        hit = (top2 == e).any(axis=1)
        idx_lists.append(np.nonzero(hit)[0].astype(np.int64))

    # 16 blocks = (expert, F-half); rank by token count, big 8 are A slots
    blocks = [(e, h) for e in range(E) for h in range(2)]
    blocks.sort(key=lambda b: -len(idx_lists[b[0]]))
    A_blocks = blocks[:8]
    B_blocks = blocks[8:]
    CA = max(128, _roundup(max(len(idx_lists[e]) for e, _ in A_blocks), 4))
    CB = max(128, _roundup(max(len(idx_lists[e]) for e, _ in B_blocks), 4))

    xT = np.ascontiguousarray(xf.T)
    ident = np.eye(128, dtype=np.float32)
    centT_packed = _pack_xT(np.ascontiguousarray(centroids.T), E)
    FH2 = F // 2

    def rank_masks(ix, C, expert):
        """Host dispatch metadata: top-1/top-2 one-hot masks + is-top-1
        flag for this slot's tokens, laid out [row=tok%128, tile=tok//128]."""
        nt = len(_jtiles(C))
        m1 = np.zeros((128, nt, E), np.float32)
        m2 = np.zeros((128, nt, E), np.float32)
        r1 = np.zeros((128, nt), np.float32)
        for p, t in enumerate(ix):
            row, tile = p % 128, p // 128
            m1[row, tile, top2[t, 0]] = 1.0
            m2[row, tile, top2[t, 1]] = 1.0
            if top2[t, 0] == expert:
                r1[row, tile] = 1.0
        return (
            np.ascontiguousarray(m1.reshape(128, nt * E)),
            np.ascontiguousarray(m2.reshape(128, nt * E)),
            r1,
        )

    in_maps = []
    for c in range(E):
        eA, hA = A_blocks[c]
        eB, hB = B_blocks[c]
        ixA = idx_lists[eA]
        ixB = idx_lists[eB]
        m1a, m2a, r1a = rank_masks(ixA, CA, eA)
        m1b, m2b, r1b = rank_masks(ixB, CB, eB)
        misc = np.concatenate([ident, m1a, m2a, r1a, m1b, m2b, r1b], axis=1)
        misc = np.ascontiguousarray(misc, np.float32)
        q = c // 2
        half = c % 2
        wfcsh = np.concatenate(
            [Wfc_sh[n, half * FH2:(half + 1) * FH2, :] for n in range(NS)], axis=0
        )  # [FSH, D]
        wpjsh = np.concatenate(
            [Wproj_sh[n, :, half * FH2:(half + 1) * FH2] for n in range(NS)], axis=1
        )  # [D, FSH]
        in_maps.append({
            "xa": _pack_xT(xT[:, ixA], CA),
            "xb": _pack_xT(xT[:, ixB], CB),
            "xq": _pack_xT(xT[:, q * TQ:(q + 1) * TQ], TQ),
            "x_res": np.ascontiguousarray(xf[q * TQ:(q + 1) * TQ] * 0.5),
            "centT": centT_packed,
            "misc": misc,
            "wfcA": _pack_lhsT(Wfc_r[eA, hA * FHALF:(hA + 1) * FHALF, :].T),
            "wpjA": _pack_rhs(Wproj_r[eA, :, hA * FHALF:(hA + 1) * FHALF].T),
            "wfcB": _pack_lhsT(Wfc_r[eB, hB * FHALF:(hB + 1) * FHALF, :].T),
            "wpjB": _pack_rhs(Wproj_r[eB, :, hB * FHALF:(hB + 1) * FHALF].T),
            "wfcS": _pack_lhsT(np.ascontiguousarray(wfcsh.T)),
            "wpjS": _pack_rhs(np.ascontiguousarray(wpjsh.T)),
        })
    meta = (A_blocks, B_blocks, CA, CB, idx_lists)
    return in_maps, meta


def _assemble(results, meta):
    A_blocks, B_blocks, CA, CB, idx_lists = meta
    out = np.zeros((TT, D), np.float32)
    for c in range(E):
        r = results[c]
        q = c // 2
        out[q * TQ:(q + 1) * TQ] += np.asarray(r["outS"]).astype(np.float32)
        eA, _ = A_blocks[c]
        eB, _ = B_blocks[c]
        ixA = idx_lists[eA]
        ixB = idx_lists[eB]
        out[ixA] += np.asarray(r["outA"])[:len(ixA)].astype(np.float32)
        out[ixB] += np.asarray(r["outB"])[:len(ixB)].astype(np.float32)
    return out.reshape(B, S, D)


def kernel(x, centroids, routing_bias, Wfc_r, Wproj_r, Wfc_sh, Wproj_sh):
    from concourse import bass_utils

    in_maps, meta = _prepare(
        x, centroids, routing_bias, Wfc_r, Wproj_r, Wfc_sh, Wproj_sh
    )
    CA, CB = meta[2], meta[3]
    if (CA, CB) not in _prog_cache:
        _prog_cache[(CA, CB)] = _build(CA, CB)
    nc = _prog_cache[(CA, CB)]

    globals()["_last_in_maps"] = in_maps
    globals()["_last_meta"] = meta
    res = bass_utils.run_bass_kernel_spmd(nc, in_maps, core_ids=list(range(E)))
    globals()["_last_results"] = res
    return _assemble(res.results, meta)
